# revision 18
# baseline (speedup 1.0000x reference)
"""Trainium2 Bass kernel for nn_EvolvingSystem (moe_routing).

Math (reference):
  psi = softmax_c(-d2),  d2[b,c] = ||si_c^T(mu_c - z_b)||^2
  ARX: preds[b,c,l] from linear recursion on state0 = y[:,:,-16:] and
       ub[b,c] = u[b,c,:].b_coef[c] + bias[c]
  out[b,l] = sum_c psi[b,c] preds[b,c,l]

Device strategy (8 cores, data-parallel on B, 1024 rows/core):
  d2[b,c] = ||t_bc||^2 - 2 z_b.q_c + k_c   with t_bc = si_c^T z_b,
  q_c = si_c si_c^T mu_c, k_c = ||si_c^T mu_c||^2 (host-precomputed).
  T = Z @ si_c: fp32r matmuls at full PE rate, one [128,512] PSUM tile
  per (cluster-pair, batch-chunk).  Batch-chunk (bk) is the OUTER loop
  with all 16 sigma tiles resident, so each bk's softmax chain overlaps
  the following bk's matmuls.  Square-reduce consumers: ACT pairs use
  activation(Square, accum_out), DVE pairs use bn_stats on a
  cluster-interleaved layout (sumsq = M2 + 256*mean^2 fixup).
  Per-bk endchain: d2 assembly, Exp with constant bias (d2 in [55,145]
  so exp(90-d2) is safe fp32; no row-max needed), segmented softmax,
  bf16 PE transpose into one PSUM bank.  The ARX tail runs per 512-row
  half right after bk3/bk7.
  ARX recursion is linear -> host-unrolled coefficients W[c,l,o], g[c,l]:
    preds[b,c,l] = sum_o W[c,l,o] state0[b,c,o] + g[c,l] ub[b,c]
    out^T[l,b] = Wflat^T @ (psi*state0)^T + g^T @ (psi*ub)^T  (small matmuls)

DMA: small parameters are packed into a few merged loads (each DMA
costs ~0.6us of queue time regardless of size); startup-critical zt /
sigma tiles are spread across the three DMA queues (SP + ACT HWDGE,
gpsimd SWDGE); the ACT queue finishes all its DMA issues before the
main loop needs it for Square consumers.
"""

import sys
from contextlib import ExitStack

import numpy as np

if "/opt/trn_rl_repo" not in sys.path:
    sys.path.insert(0, "/opt/trn_rl_repo")

import ml_dtypes

import concourse.bass as bass
import concourse.mybir as mybir
import concourse.tile as tile
from concourse import bacc
from concourse.bass_utils import run_bass_kernel_spmd

N_CORES = 8
B, C, D = 8192, 16, 256
R, E, ORD, L = 64, 32, 16, 32
BLOC = B // N_CORES            # 1024
NBK = BLOC // 128              # 8 batch chunks of 128
CE = C * E                     # 512
CO = C * ORD                   # 256
NPAIR = C // 2                 # 8 cluster pairs
ACT_PAIRS = (0, 4, 6)          # squared on ACT (contiguous layout)
DVE_PAIRS = (1, 2, 3, 5, 7)    # bn_stats on DVE (interleaved layout)
EXPB = 90.0                    # exp(EXPB - d2); d2 in [55, 145]

F32 = mybir.dt.float32
F32R = mybir.dt.float32r
BF16 = mybir.dt.bfloat16

_CACHE = {}


def build_program():
    nc = bacc.Bacc(
        "TRN2",
        target_bir_lowering=False,
        debug=False,
        enable_asserts=False,
        num_devices=N_CORES,
    )

    # ---- DRAM I/O (per-core shapes) ----
    zt_d = nc.dram_tensor("zt", [D, BLOC], F32R, kind="ExternalInput").ap()
    # sgr[i, pair, :]: ACT pairs contiguous halves, DVE pairs interleaved
    sgr_d = nc.dram_tensor("sgr", [D, NPAIR, 512], F32R, kind="ExternalInput").ap()
    # pk1: qa0 | qa1 | ktile16 | ebias  (f32, bitcast to f32r where needed)
    pk1_d = nc.dram_tensor("pk1", [128, 97], F32R, kind="ExternalInput").ap()
    # pkw: wf0 | wf1 cols (wflat k-chunks), bmp: bmat k-chunks
    pkw_d = nc.dram_tensor("pkw", [128, 2 * L], F32R, kind="ExternalInput").ap()
    bmp_d = nc.dram_tensor("bmp", [128, 4 * C], F32R, kind="ExternalInput").ap()
    # pke: emat | gmat | biasv  (16-partition params)
    pke_d = nc.dram_tensor("pke", [C, CO + L + 1], F32R, kind="ExternalInput").ap()
    s0t_d = nc.dram_tensor("s0t", [CO, BLOC], F32, kind="ExternalInput").ap()
    ut_d = nc.dram_tensor("ut", [CE, BLOC], F32R, kind="ExternalInput").ap()
    identb_d = nc.dram_tensor("identb", [128, 128], BF16, kind="ExternalInput").ap()
    out_d = nc.dram_tensor("outT", [L, BLOC], F32, kind="ExternalOutput").ap()

    with tile.TileContext(nc) as tc, ExitStack() as ctx:
        const = ctx.enter_context(tc.tile_pool(name="const", bufs=1))
        scr = ctx.enter_context(tc.tile_pool(name="scr", bufs=3))
        soft = ctx.enter_context(tc.tile_pool(name="soft", bufs=4))
        tailp = ctx.enter_context(tc.tile_pool(name="tailp", bufs=4))
        ps_t = ctx.enter_context(tc.tile_pool(name="ps_t", bufs=6, space="PSUM"))
        ps_dots = ctx.enter_context(tc.tile_pool(name="ps_dots", bufs=1, space="PSUM"))
        ps_pt = ctx.enter_context(tc.tile_pool(name="ps_pt", bufs=1, space="PSUM"))

        # ---- startup loads, spread across the three DMA queues ----
        # sync: zt k0,k1 then sgr p4,p5 then pk1, s0t, ut
        # scalar: zt k2,k3 then sgr p6,p7 then small params (done early)
        # gpsimd: sgr p0..p3
        zt0c, zt1c = [], []
        for k in range(4):
            cs = slice(k * 256, (k + 1) * 256)
            t0 = const.tile([128, 256], F32R, tag=f"zt0c{k}", name=f"zt0c{k}")
            t1 = const.tile([128, 256], F32R, tag=f"zt1c{k}", name=f"zt1c{k}")
            eng = nc.sync if k < 2 else nc.scalar
            eng.dma_start(t0[:], zt_d[0:128, cs])
            eng.dma_start(t1[:], zt_d[128:256, cs])
            zt0c.append(t0)
            zt1c.append(t1)

        def zt0s(bk):
            return zt0c[bk // 2][:, (bk % 2) * 128 : (bk % 2) * 128 + 128]

        def zt1s(bk):
            return zt1c[bk // 2][:, (bk % 2) * 128 : (bk % 2) * 128 + 128]

        sg0, sg1 = [None] * NPAIR, [None] * NPAIR
        for p in range(NPAIR):
            sg0[p] = const.tile([128, 512], F32R, tag=f"sg0_{p}", name=f"sg0_{p}")
            sg1[p] = const.tile([128, 512], F32R, tag=f"sg1_{p}", name=f"sg1_{p}")
        pk1 = const.tile([128, 97], F32R, tag="pk1", name="pk1")
        qa0 = pk1[:, 0:16]
        qa1 = pk1[:, 16:32]
        ktile64 = pk1[:, 32:96]
        ebias = pk1[:, 96:97]
        identb = const.tile([128, 128], BF16, tag="identb", name="identb")
        pkw = const.tile([128, 2 * L], F32R, tag="pkw", name="pkw")
        wf0 = pkw[:, 0:L]
        wf1 = pkw[:, L : 2 * L]
        bmp = const.tile([128, 4 * C], F32R, tag="bmp", name="bmp")
        pke = const.tile([C, CO + L + 1], F32R, tag="pke", name="pke")
        emat = pke[:, 0:CO]
        gmat = pke[:, CO : CO + L]
        biasv = pke[:, CO + L : CO + L + 1]
        s0t = [
            const.tile([128, BLOC], F32, tag=f"s0t{k}", name=f"s0t{k}")
            for k in range(2)
        ]
        ut = [
            const.tile([128, BLOC], F32R, tag=f"ut{k}", name=f"ut{k}")
            for k in range(4)
        ]

        # gpsimd: pair0 in 256-col chunks (earliest), then pair3
        for cs in (slice(0, 256), slice(256, 512)):
            nc.gpsimd.dma_start(sg0[0][:, cs], sgr_d[0:128, 0, cs])
            nc.gpsimd.dma_start(sg1[0][:, cs], sgr_d[128:256, 0, cs])
        nc.gpsimd.dma_start(sg0[3][:], sgr_d[0:128, 3, :])
        nc.gpsimd.dma_start(sg1[3][:], sgr_d[128:256, 3, :])
        # sync (zt k0,k1 queued above): pair1, pk1, pairs 4,5, params,
        # pairs 6,7, s0t, ut.  scalar (zt k2,k3 above): pair2 below.
        nc.sync.dma_start(sg0[1][:], sgr_d[0:128, 1, :])
        nc.sync.dma_start(sg1[1][:], sgr_d[128:256, 1, :])
        nc.sync.dma_start(pk1[:], pk1_d[:])
        for p in (4, 5):
            nc.sync.dma_start(sg0[p][:], sgr_d[0:128, p, :])
            nc.sync.dma_start(sg1[p][:], sgr_d[128:256, p, :])
        nc.sync.dma_start(identb[:], identb_d[:])
        nc.sync.dma_start(pkw[:], pkw_d[:])
        nc.sync.dma_start(bmp[:], bmp_d[:])
        nc.sync.dma_start(pke[:], pke_d[:])
        for p in (6, 7):
            nc.sync.dma_start(sg0[p][:], sgr_d[0:128, p, :])
            nc.sync.dma_start(sg1[p][:], sgr_d[128:256, p, :])
        for k in range(2):
            nc.sync.dma_start(s0t[k][:], s0t_d[k * 128 : (k + 1) * 128, :])
        for k in range(4):
            nc.sync.dma_start(ut[k][:], ut_d[k * 128 : (k + 1) * 128, :])
        # scalar (zt k2,k3 already queued above): pair2 only; ACT free early
        nc.scalar.dma_start(sg0[2][:], sgr_d[0:128, 2, :])
        nc.scalar.dma_start(sg1[2][:], sgr_d[128:256, 2, :])

        # ---- main loop: bk outer, pairs inner; per-bk softmax ----
        dots = ps_dots.tile([128, 128], F32, tag="dots", name="dots")
        sqacc = const.tile([128, 128], F32, tag="sqacc", name="sqacc")
        pt_all = ps_pt.tile([C, BLOC], BF16, tag="pt", name="pt")
        psit_r = const.tile([C, BLOC], F32R, tag="psit_r", name="psit_r")
        nd = len(DVE_PAIRS)
        dve_slot = {p: i for i, p in enumerate(DVE_PAIRS)}
        ubps = [None, None]

        def tail_half(bh):
            bsl = slice(bh * 512, (bh + 1) * 512)
            psie = []
            for k in range(2):
                p = ps_t.tile([128, 512], F32, tag="t_ps", name="tail")
                nc.tensor.matmul(
                    p[:],
                    emat[:, k * 128 : (k + 1) * 128],
                    psit_r[:, bsl],
                    start=True,
                    stop=True,
                )
                psie.append(p)
            a_sb = []
            for k in range(2):
                t = tailp.tile([128, 512], F32R, tag="a_sb", name="a_sb")
                nc.vector.tensor_tensor(
                    t[:], s0t[k][:, bsl], psie[k][:], op=mybir.AluOpType.mult
                )
                a_sb.append(t)
            ubp = ubps[bh]
            pt_sb = tailp.tile([C, 512], F32R, tag="pt_sb", name="pt_sb")
            nc.vector.scalar_tensor_tensor(
                out=pt_sb[:],
                in0=ubp[:],
                scalar=biasv,
                in1=psit_r[:, bsl],
                op0=mybir.AluOpType.add,
                op1=mybir.AluOpType.mult,
            )
            outp = ps_t.tile([L, 512], F32, tag="t_ps", name="tail")
            nc.tensor.matmul(outp[:], wf0, a_sb[0][:], start=True, stop=False)
            nc.tensor.matmul(outp[:], wf1, a_sb[1][:], start=False, stop=False)
            nc.tensor.matmul(outp[:], gmat, pt_sb[:], start=False, stop=True)
            out_sb = tailp.tile([L, 512], F32, tag="out_sb", name="out_sb")
            nc.vector.tensor_copy(out_sb[:], outp[:])
            nc.sync.dma_start(out_d[:, bsl], out_sb[:])

        stats = const.tile([128, NBK, nd, 6], F32, tag="stats", name="stats")

        def consume(pair, bk, t_ps):
            base = bk * C
            if pair in ACT_PAIRS:
                for cc in range(2):
                    c = 2 * pair + cc
                    o = scr.tile([128, 256], F32, tag="scr", name="scr")
                    nc.scalar.activation(
                        o[:],
                        t_ps[:, cc * 256 : (cc + 1) * 256],
                        mybir.ActivationFunctionType.Square,
                        accum_out=sqacc[:, base + c : base + c + 1],
                    )
            else:
                nc.vector.bn_stats(stats[:, bk, dve_slot[pair], :], t_ps[:])

        def pair_mm(pair, bk):
            t_ps = ps_t.tile([128, 512], F32, tag="t_ps", name="t_ps")
            nc.tensor.matmul(t_ps[:], zt0s(bk), sg0[pair][:], start=True, stop=False)
            nc.tensor.matmul(t_ps[:], zt1s(bk), sg1[pair][:], start=False, stop=True)
            consume(pair, bk, t_ps)

        # phase 1: early-arriving pairs 0..3, pair-major across all bks
        for pair in range(4):
            for bk in range(NBK):
                if pair == 0:
                    dsl = dots[:, bk * C : (bk + 1) * C]
                    nc.tensor.matmul(dsl, zt0s(bk), qa0, start=True, stop=False)
                    nc.tensor.matmul(dsl, zt1s(bk), qa1, start=False, stop=True)
                pair_mm(pair, bk)

        # phase 2: pairs 4..7 per bk; fused endchain + tail per 512-half
        def endchain_bks(b0, b1):
            csl = slice(b0 * C, b1 * C)
            nbk = b1 - b0
            # fixup: sumsq = M2 + 256*mean^2 for bks [b0, b1) at once
            stv = stats[:, b0:b1]
            v_mu = stv[:, :, :, 1:6:3]         # [128, 4, nd, 2]
            v_m2 = stv[:, :, :, 2:6:3]
            tmp = soft.tile([128, nbk, nd, 2], F32, tag="fix", name="fix")
            nc.vector.tensor_tensor(tmp[:], v_mu, v_mu, op=mybir.AluOpType.mult)
            sqv = sqacc[:, csl].rearrange("p (g x) -> p g x", x=C)
            # DVE pairs (1,2,3),(5),(7) -> col blocks 2:8, 10:12, 14:16
            for slots, c0, c1 in ((slice(0, 3), 2, 8), (slice(3, 4), 10, 12),
                                  (slice(4, 5), 14, 16)):
                ov = sqv[:, :, c0:c1].rearrange("p g (s t) -> p g s t", t=2)
                nc.vector.scalar_tensor_tensor(
                    out=ov,
                    in0=tmp[:, :, slots, :],
                    scalar=256.0,
                    in1=v_m2[:, :, slots, :],
                    op0=mybir.AluOpType.mult,
                    op1=mybir.AluOpType.add,
                )
            d2a = soft.tile([128, nbk * C], F32, tag="d2a", name="d2a")
            nc.vector.scalar_tensor_tensor(
                out=d2a[:],
                in0=dots[:, csl],
                scalar=-2.0,
                in1=sqacc[:, csl],
                op0=mybir.AluOpType.mult,
                op1=mybir.AluOpType.add,
            )
            d2t = soft.tile([128, nbk * C], F32, tag="d2t", name="d2t")
            nc.vector.tensor_tensor(
                d2t[:], d2a[:], ktile64[:, : nbk * C], op=mybir.AluOpType.add
            )
            et = soft.tile([128, nbk * C], F32, tag="et", name="et")
            nc.scalar.activation(
                et[:],
                d2t[:],
                mybir.ActivationFunctionType.Exp,
                bias=ebias,
                scale=-1.0,
            )
            den4 = soft.tile([128, nbk], F32, tag="den", name="den")
            nc.vector.tensor_reduce(
                den4[:],
                et[:].rearrange("p (g x) -> p g x", x=C),
                axis=mybir.AxisListType.X,
                op=mybir.AluOpType.add,
            )
            rden4 = soft.tile([128, nbk], F32, tag="rden", name="rden")
            nc.vector.reciprocal(rden4[:], den4[:])
            psi = soft.tile([128, nbk * C], BF16, tag="psi", name="psi")
            for g in range(nbk):
                nc.vector.tensor_scalar_mul(
                    psi[:, g * C : (g + 1) * C],
                    et[:, g * C : (g + 1) * C],
                    rden4[:, g : g + 1],
                )
            return psi

        def transposes_for(psi, b0, b1):
            for g in range(b1 - b0):
                bk = b0 + g
                nc.tensor.transpose(
                    pt_all[:, bk * 128 : (bk + 1) * 128],
                    psi[:, g * C : (g + 1) * C],
                    identb[:],
                )

        def hoist_ubp(bh):
            bsl = slice(bh * 512, (bh + 1) * 512)
            ubp = ps_t.tile([C, 512], F32, tag="t_ps", name="tail")
            for k in range(4):
                nc.tensor.matmul(
                    ubp[:],
                    bmp[:, k * C : (k + 1) * C],
                    ut[k][:, bsl],
                    start=(k == 0),
                    stop=(k == 3),
                )
            ubps[bh] = ubp

        def copy_psit(bh):
            bsl = slice(bh * 512, (bh + 1) * 512)
            nc.scalar.activation(
                psit_r[:, bsl],
                pt_all[:, bsl],
                mybir.ActivationFunctionType.Copy,
            )

        psis = {}
        for bk in range(NBK):
            for pair in range(4, NPAIR):
                pair_mm(pair, bk)
            if bk == 2:
                hoist_ubp(0)
            if bk == 3:
                # DVE part right after bk3 stats; PE transposes after bk4's
                # mms so neither engine stalls the other
                psis[0] = endchain_bks(0, 4)
            if bk == 4:
                transposes_for(psis[0], 0, 4)
                copy_psit(0)
                tail_half(0)
            if bk == 6:
                hoist_ubp(1)
                psis[1] = endchain_bks(4, 7)
        psis[2] = endchain_bks(7, 8)
        transposes_for(psis[1], 4, 7)
        transposes_for(psis[2], 7, 8)
        copy_psit(1)
        tail_half(1)

    nc.compile()
    return nc


def host_prep(y, z, u, mu, sigma_inv, a_coef, b_coef, bias):
    """Host-side precompute: shared tensors + per-core input maps."""
    f64 = np.float64
    W = np.zeros((C, L, ORD), f64)
    g = np.zeros((C, L), f64)
    for c in range(C):
        a = a_coef[c].astype(f64)
        S = np.eye(ORD, dtype=f64)
        sb = np.zeros(ORD, f64)
        for l in range(L):
            ya = a @ S
            yb = a @ sb + 1.0
            W[c, l] = ya
            g[c, l] = yb
            S = np.vstack([S[1:], ya[None]])
            sb = np.concatenate([sb[1:], [yb]])
    wflat = np.ascontiguousarray(W.transpose(0, 2, 1).reshape(CO, L)).astype(np.float32)
    gmat = g.astype(np.float32)

    si = sigma_inv.astype(f64)
    m = np.einsum("cij,ci->cj", si, mu.astype(f64))   # p_c = si_c^T mu_c
    q = np.einsum("cij,cj->ci", si, m)                # q_c = si_c p_c
    k = np.sum(m * m, axis=1)                         # k_c = ||p_c||^2
    qa = q.T.astype(np.float32)                       # [D, C]

    # pk1: qa0 | qa1 | ktile64 | ebias
    pk1 = np.empty((128, 97), np.float32)
    pk1[:, 0:16] = qa[0:128]
    pk1[:, 16:32] = qa[128:256]
    pk1[:, 32:96] = np.tile(k.astype(np.float32), 4)[None, :]
    pk1[:, 96] = EXPB

    # pkw: wflat k-chunks side by side
    pkw = np.empty((128, 2 * L), np.float32)
    pkw[:, 0:L] = wflat[0:128]
    pkw[:, L : 2 * L] = wflat[128:256]

    # bmp: bmat k-chunks side by side
    bmat = np.zeros((CE, C), np.float32)
    for c in range(C):
        bmat[c * E : (c + 1) * E, c] = b_coef[c]
    bmp = np.empty((128, 4 * C), np.float32)
    for kk in range(4):
        bmp[:, kk * C : (kk + 1) * C] = bmat[kk * 128 : (kk + 1) * 128]

    # pke: emat | gmat | biasv
    emat = np.zeros((C, CO), np.float32)
    for c in range(C):
        emat[c, c * ORD : (c + 1) * ORD] = 1.0
    pke = np.empty((C, CO + L + 1), np.float32)
    pke[:, 0:CO] = emat
    pke[:, CO : CO + L] = gmat
    pke[:, CO + L] = bias.astype(np.float32)

    # sgr[i, pair, :]: ACT pairs store [sig_{2p} | sig_{2p+1}] contiguously,
    # DVE pairs interleave the two clusters' columns (2j+cc) for bn_stats.
    sit = sigma_inv.astype(np.float32).transpose(1, 0, 2)    # [i, c, j]
    sgr = np.empty((D, NPAIR, 512), np.float32)
    for p in range(NPAIR):
        if p in ACT_PAIRS:
            sgr[:, p, 0:256] = sit[:, 2 * p, :]
            sgr[:, p, 256:512] = sit[:, 2 * p + 1, :]
        else:
            sgr[:, p, 0::2] = sit[:, 2 * p, :]
            sgr[:, p, 1::2] = sit[:, 2 * p + 1, :]

    shared = {
        "sgr": sgr,
        "pk1": pk1,
        "pkw": pkw,
        "bmp": bmp,
        "pke": pke,
        "identb": np.eye(128, dtype=ml_dtypes.bfloat16),
    }
    in_maps = []
    for i in range(N_CORES):
        s = slice(i * BLOC, (i + 1) * BLOC)
        m_i = dict(shared)
        m_i["zt"] = np.ascontiguousarray(z[s, 0, :].T)
        m_i["s0t"] = np.ascontiguousarray(y[s, :, R - ORD :].reshape(BLOC, CO).T)
        m_i["ut"] = np.ascontiguousarray(u[s].reshape(BLOC, CE).T)
        in_maps.append(m_i)
    return in_maps


def kernel(y, z, u, mu, sigma_inv, a_coef, b_coef, bias, _trace=False):
    if "nc" not in _CACHE:
        _CACHE["nc"] = build_program()
    nc = _CACHE["nc"]
    in_maps = host_prep(y, z, u, mu, sigma_inv, a_coef, b_coef, bias)
    res = run_bass_kernel_spmd(
        nc, in_maps, core_ids=list(range(N_CORES)), trace=_trace
    )
    _CACHE["last_result"] = res
    out = np.concatenate(
        [res.results[i]["outT"].T[:, None, :] for i in range(N_CORES)], axis=0
    )
    return out


# revision 19
# speedup vs baseline: 1.0263x; 1.0263x over previous
"""Trainium2 Bass kernel for nn_EvolvingSystem (moe_routing).

Math (reference):
  psi = softmax_c(-d2),  d2[b,c] = ||si_c^T(mu_c - z_b)||^2
  ARX: preds[b,c,l] from linear recursion on state0 = y[:,:,-16:] and
       ub[b,c] = u[b,c,:].b_coef[c] + bias[c]
  out[b,l] = sum_c psi[b,c] preds[b,c,l]

Device strategy (8 cores, data-parallel on B, 1024 rows/core):
  d2[b,c] = ||t_bc||^2 - 2 z_b.q_c + k_c   with t_bc = si_c^T z_b,
  q_c = si_c si_c^T mu_c, k_c = ||si_c^T mu_c||^2 (host-precomputed).
  T = Z @ si_c: fp32r matmuls at full PE rate, one [128,512] PSUM tile
  per (cluster-pair, batch-chunk).  Batch-chunk (bk) is the OUTER loop
  with all 16 sigma tiles resident, so each bk's softmax chain overlaps
  the following bk's matmuls.  Square-reduce consumers: ACT pairs use
  activation(Square, accum_out), DVE pairs use bn_stats on a
  cluster-interleaved layout (sumsq = M2 + 256*mean^2 fixup).
  Per-bk endchain: d2 assembly, Exp with constant bias (d2 in [55,145]
  so exp(90-d2) is safe fp32; no row-max needed), segmented softmax,
  bf16 PE transpose into one PSUM bank.  The ARX tail runs per 512-row
  half right after bk3/bk7.
  ARX recursion is linear -> host-unrolled coefficients W[c,l,o], g[c,l]:
    preds[b,c,l] = sum_o W[c,l,o] state0[b,c,o] + g[c,l] ub[b,c]
    out^T[l,b] = Wflat^T @ (psi*state0)^T + g^T @ (psi*ub)^T  (small matmuls)

DMA: small parameters are packed into a few merged loads (each DMA
costs ~0.6us of queue time regardless of size); startup-critical zt /
sigma tiles are spread across the three DMA queues (SP + ACT HWDGE,
gpsimd SWDGE); the ACT queue finishes all its DMA issues before the
main loop needs it for Square consumers.
"""

import sys
from contextlib import ExitStack

import numpy as np

if "/opt/trn_rl_repo" not in sys.path:
    sys.path.insert(0, "/opt/trn_rl_repo")

import ml_dtypes

import concourse.bass as bass
import concourse.mybir as mybir
import concourse.tile as tile
from concourse import bacc
from concourse.bass_utils import run_bass_kernel_spmd

N_CORES = 8
B, C, D = 8192, 16, 256
R, E, ORD, L = 64, 32, 16, 32
BLOC = B // N_CORES            # 1024
NBK = BLOC // 128              # 8 batch chunks of 128
CE = C * E                     # 512
CO = C * ORD                   # 256
NPAIR = C // 2                 # 8 cluster pairs
ACT_PAIRS = (0, 4, 6)          # squared on ACT (contiguous layout)
DVE_PAIRS = (1, 2, 3, 5, 7)    # bn_stats on DVE (interleaved layout)
EXPB = 90.0                    # exp(EXPB - d2); d2 in [55, 145]

F32 = mybir.dt.float32
F32R = mybir.dt.float32r
BF16 = mybir.dt.bfloat16

_CACHE = {}


def build_program():
    nc = bacc.Bacc(
        "TRN2",
        target_bir_lowering=False,
        debug=False,
        enable_asserts=False,
        num_devices=N_CORES,
    )

    # ---- DRAM I/O (per-core shapes) ----
    zt_d = nc.dram_tensor("zt", [D, BLOC], F32R, kind="ExternalInput").ap()
    # sgr[i, pair, :]: ACT pairs contiguous halves, DVE pairs interleaved
    sgr_d = nc.dram_tensor("sgr", [D, NPAIR, 512], F32R, kind="ExternalInput").ap()
    # pk1: qa0 | qa1 | ktile16 | ebias  (f32, bitcast to f32r where needed)
    pk1_d = nc.dram_tensor("pk1", [128, 97], F32R, kind="ExternalInput").ap()
    # pkw: wf0 | wf1 cols (wflat k-chunks), bmp: bmat k-chunks
    pkw_d = nc.dram_tensor("pkw", [128, 2 * L], F32R, kind="ExternalInput").ap()
    bmp_d = nc.dram_tensor("bmp", [128, 4 * C], F32R, kind="ExternalInput").ap()
    # pke: emat | gmat | biasv  (16-partition params)
    pke_d = nc.dram_tensor("pke", [C, CO + L + 1], F32R, kind="ExternalInput").ap()
    s0t_d = nc.dram_tensor("s0t", [CO, BLOC], F32, kind="ExternalInput").ap()
    ut_d = nc.dram_tensor("ut", [CE, BLOC], F32R, kind="ExternalInput").ap()
    identb_d = nc.dram_tensor("identb", [128, 128], BF16, kind="ExternalInput").ap()
    out_d = nc.dram_tensor("outT", [L, BLOC], F32, kind="ExternalOutput").ap()

    with tile.TileContext(nc) as tc, ExitStack() as ctx:
        const = ctx.enter_context(tc.tile_pool(name="const", bufs=1))
        scr = ctx.enter_context(tc.tile_pool(name="scr", bufs=3))
        soft = ctx.enter_context(tc.tile_pool(name="soft", bufs=4))
        tailp = ctx.enter_context(tc.tile_pool(name="tailp", bufs=4))
        ps_t = ctx.enter_context(tc.tile_pool(name="ps_t", bufs=6, space="PSUM"))
        ps_dots = ctx.enter_context(tc.tile_pool(name="ps_dots", bufs=1, space="PSUM"))
        ps_pt = ctx.enter_context(tc.tile_pool(name="ps_pt", bufs=1, space="PSUM"))

        # ---- startup loads, spread across the three DMA queues ----
        # sync: zt k0,k1 then sgr p4,p5 then pk1, s0t, ut
        # scalar: zt k2,k3 then sgr p6,p7 then small params (done early)
        # gpsimd: sgr p0..p3
        zt0c, zt1c = [], []
        for k in range(4):
            cs = slice(k * 256, (k + 1) * 256)
            t0 = const.tile([128, 256], F32R, tag=f"zt0c{k}", name=f"zt0c{k}")
            t1 = const.tile([128, 256], F32R, tag=f"zt1c{k}", name=f"zt1c{k}")
            eng = nc.sync if k < 2 else nc.scalar
            eng.dma_start(t0[:], zt_d[0:128, cs])
            eng.dma_start(t1[:], zt_d[128:256, cs])
            zt0c.append(t0)
            zt1c.append(t1)

        def zt0s(bk):
            return zt0c[bk // 2][:, (bk % 2) * 128 : (bk % 2) * 128 + 128]

        def zt1s(bk):
            return zt1c[bk // 2][:, (bk % 2) * 128 : (bk % 2) * 128 + 128]

        sg0, sg1 = [None] * NPAIR, [None] * NPAIR
        for p in range(NPAIR):
            sg0[p] = const.tile([128, 512], F32R, tag=f"sg0_{p}", name=f"sg0_{p}")
            sg1[p] = const.tile([128, 512], F32R, tag=f"sg1_{p}", name=f"sg1_{p}")
        pk1 = const.tile([128, 97], F32R, tag="pk1", name="pk1")
        qa0 = pk1[:, 0:16]
        qa1 = pk1[:, 16:32]
        ktile64 = pk1[:, 32:96]
        ebias = pk1[:, 96:97]
        identb = const.tile([128, 128], BF16, tag="identb", name="identb")
        pkw = const.tile([128, 2 * L], F32R, tag="pkw", name="pkw")
        wf0 = pkw[:, 0:L]
        wf1 = pkw[:, L : 2 * L]
        bmp = const.tile([128, 4 * C], F32R, tag="bmp", name="bmp")
        pke = const.tile([C, CO + L + 1], F32R, tag="pke", name="pke")
        emat = pke[:, 0:CO]
        gmat = pke[:, CO : CO + L]
        biasv = pke[:, CO + L : CO + L + 1]
        s0t = [
            const.tile([128, BLOC], F32, tag=f"s0t{k}", name=f"s0t{k}")
            for k in range(2)
        ]
        ut = [
            const.tile([128, BLOC], F32R, tag=f"ut{k}", name=f"ut{k}")
            for k in range(4)
        ]

        # gpsimd: pair0 in 256-col chunks (earliest), then pair3
        for cs in (slice(0, 256), slice(256, 512)):
            nc.gpsimd.dma_start(sg0[0][:, cs], sgr_d[0:128, 0, cs])
            nc.gpsimd.dma_start(sg1[0][:, cs], sgr_d[128:256, 0, cs])
        nc.gpsimd.dma_start(sg0[3][:], sgr_d[0:128, 3, :])
        nc.gpsimd.dma_start(sg1[3][:], sgr_d[128:256, 3, :])
        # sync (zt k0,k1 queued above): pair1, pk1, pairs 4,5, params,
        # pairs 6,7, s0t, ut.  scalar (zt k2,k3 above): pair2 below.
        nc.sync.dma_start(sg0[1][:], sgr_d[0:128, 1, :])
        nc.sync.dma_start(sg1[1][:], sgr_d[128:256, 1, :])
        nc.sync.dma_start(pk1[:], pk1_d[:])
        for p in (4, 5):
            nc.sync.dma_start(sg0[p][:], sgr_d[0:128, p, :])
            nc.sync.dma_start(sg1[p][:], sgr_d[128:256, p, :])
        nc.sync.dma_start(identb[:], identb_d[:])
        nc.sync.dma_start(pkw[:], pkw_d[:])
        nc.sync.dma_start(bmp[:], bmp_d[:])
        nc.sync.dma_start(pke[:], pke_d[:])
        for p in (6, 7):
            nc.sync.dma_start(sg0[p][:], sgr_d[0:128, p, :])
            nc.sync.dma_start(sg1[p][:], sgr_d[128:256, p, :])
        for k in range(2):
            nc.sync.dma_start(s0t[k][:], s0t_d[k * 128 : (k + 1) * 128, :])
        for k in range(4):
            nc.sync.dma_start(ut[k][:], ut_d[k * 128 : (k + 1) * 128, :])
        # scalar (zt k2,k3 already queued above): pair2 only; ACT free early
        nc.scalar.dma_start(sg0[2][:], sgr_d[0:128, 2, :])
        nc.scalar.dma_start(sg1[2][:], sgr_d[128:256, 2, :])

        # ---- main loop: bk outer, pairs inner; per-bk softmax ----
        dots = ps_dots.tile([128, 128], F32, tag="dots", name="dots")
        sqacc = const.tile([128, 128], F32, tag="sqacc", name="sqacc")
        pt_all = ps_pt.tile([C, BLOC], BF16, tag="pt", name="pt")
        psit_r = const.tile([C, BLOC], F32R, tag="psit_r", name="psit_r")
        nd = len(DVE_PAIRS)
        dve_slot = {p: i for i, p in enumerate(DVE_PAIRS)}
        ubps = [None, None]

        def tail_half(bh):
            bsl = slice(bh * 512, (bh + 1) * 512)
            psie = []
            for k in range(2):
                p = ps_t.tile([128, 512], F32, tag="t_ps", name="tail")
                nc.tensor.matmul(
                    p[:],
                    emat[:, k * 128 : (k + 1) * 128],
                    psit_r[:, bsl],
                    start=True,
                    stop=True,
                )
                psie.append(p)
            a_sb = []
            for k in range(2):
                t = tailp.tile([128, 512], F32R, tag="a_sb", name="a_sb")
                nc.vector.tensor_tensor(
                    t[:], s0t[k][:, bsl], psie[k][:], op=mybir.AluOpType.mult
                )
                a_sb.append(t)
            ubp = ubps[bh]
            pt_sb = tailp.tile([C, 512], F32R, tag="pt_sb", name="pt_sb")
            nc.vector.scalar_tensor_tensor(
                out=pt_sb[:],
                in0=ubp[:],
                scalar=biasv,
                in1=psit_r[:, bsl],
                op0=mybir.AluOpType.add,
                op1=mybir.AluOpType.mult,
            )
            outp = ps_t.tile([L, 512], F32, tag="t_ps", name="tail")
            nc.tensor.matmul(outp[:], wf0, a_sb[0][:], start=True, stop=False)
            nc.tensor.matmul(outp[:], wf1, a_sb[1][:], start=False, stop=False)
            nc.tensor.matmul(outp[:], gmat, pt_sb[:], start=False, stop=True)
            out_sb = tailp.tile([L, 512], F32, tag="out_sb", name="out_sb")
            nc.vector.tensor_copy(out_sb[:], outp[:])
            nc.sync.dma_start(out_d[:, bsl], out_sb[:])

        stats = const.tile([128, NBK, nd, 6], F32, tag="stats", name="stats")

        def consume(pair, bk, t_ps):
            base = bk * C
            if pair in ACT_PAIRS:
                for cc in range(2):
                    c = 2 * pair + cc
                    o = scr.tile([128, 256], F32, tag="scr", name="scr")
                    nc.scalar.activation(
                        o[:],
                        t_ps[:, cc * 256 : (cc + 1) * 256],
                        mybir.ActivationFunctionType.Square,
                        accum_out=sqacc[:, base + c : base + c + 1],
                    )
            else:
                nc.vector.bn_stats(stats[:, bk, dve_slot[pair], :], t_ps[:])

        def pair_mm(pair, bk):
            t_ps = ps_t.tile([128, 512], F32, tag="t_ps", name="t_ps")
            nc.tensor.matmul(t_ps[:], zt0s(bk), sg0[pair][:], start=True, stop=False)
            nc.tensor.matmul(t_ps[:], zt1s(bk), sg1[pair][:], start=False, stop=True)
            consume(pair, bk, t_ps)

        # phase 1: early-arriving pairs 0..3, pair-major across all bks
        for pair in range(4):
            for bk in range(NBK):
                if pair == 0:
                    dsl = dots[:, bk * C : (bk + 1) * C]
                    nc.tensor.matmul(dsl, zt0s(bk), qa0, start=True, stop=False)
                    nc.tensor.matmul(dsl, zt1s(bk), qa1, start=False, stop=True)
                pair_mm(pair, bk)

        # phase 2: pairs 4..7 per bk; fused endchain + tail per 512-half
        def endchain_bks(b0, b1):
            csl = slice(b0 * C, b1 * C)
            nbk = b1 - b0
            # fixup: sumsq = M2 + 256*mean^2 for bks [b0, b1) at once
            stv = stats[:, b0:b1]
            v_mu = stv[:, :, :, 1:6:3]         # [128, 4, nd, 2]
            v_m2 = stv[:, :, :, 2:6:3]
            tmp = soft.tile([128, nbk, nd, 2], F32, tag="fix", name="fix")
            nc.vector.tensor_tensor(tmp[:], v_mu, v_mu, op=mybir.AluOpType.mult)
            sqv = sqacc[:, csl].rearrange("p (g x) -> p g x", x=C)
            # DVE pairs (1,2,3),(5),(7) -> col blocks 2:8, 10:12, 14:16
            for slots, c0, c1 in ((slice(0, 3), 2, 8), (slice(3, 4), 10, 12),
                                  (slice(4, 5), 14, 16)):
                ov = sqv[:, :, c0:c1].rearrange("p g (s t) -> p g s t", t=2)
                nc.vector.scalar_tensor_tensor(
                    out=ov,
                    in0=tmp[:, :, slots, :],
                    scalar=256.0,
                    in1=v_m2[:, :, slots, :],
                    op0=mybir.AluOpType.mult,
                    op1=mybir.AluOpType.add,
                )
            d2a = soft.tile([128, nbk * C], F32, tag="d2a", name="d2a")
            nc.vector.scalar_tensor_tensor(
                out=d2a[:],
                in0=dots[:, csl],
                scalar=-2.0,
                in1=sqacc[:, csl],
                op0=mybir.AluOpType.mult,
                op1=mybir.AluOpType.add,
            )
            d2t = soft.tile([128, nbk * C], F32, tag="d2t", name="d2t")
            nc.vector.tensor_tensor(
                d2t[:], d2a[:], ktile64[:, : nbk * C], op=mybir.AluOpType.add
            )
            et = soft.tile([128, nbk * C], F32, tag="et", name="et")
            nc.scalar.activation(
                et[:],
                d2t[:],
                mybir.ActivationFunctionType.Exp,
                bias=ebias,
                scale=-1.0,
            )
            den4 = soft.tile([128, nbk], F32, tag="den", name="den")
            nc.vector.tensor_reduce(
                den4[:],
                et[:].rearrange("p (g x) -> p g x", x=C),
                axis=mybir.AxisListType.X,
                op=mybir.AluOpType.add,
            )
            rden4 = soft.tile([128, nbk], F32, tag="rden", name="rden")
            nc.vector.reciprocal(rden4[:], den4[:])
            psi = soft.tile([128, nbk * C], BF16, tag="psi", name="psi")
            for g in range(nbk):
                nc.vector.tensor_scalar_mul(
                    psi[:, g * C : (g + 1) * C],
                    et[:, g * C : (g + 1) * C],
                    rden4[:, g : g + 1],
                )
            return psi

        def transposes_for(psi, b0, b1):
            for g in range(b1 - b0):
                bk = b0 + g
                nc.tensor.transpose(
                    pt_all[:, bk * 128 : (bk + 1) * 128],
                    psi[:, g * C : (g + 1) * C],
                    identb[:],
                )

        def hoist_ubp(bh):
            bsl = slice(bh * 512, (bh + 1) * 512)
            ubp = ps_t.tile([C, 512], F32, tag="t_ps", name="tail")
            for k in range(4):
                nc.tensor.matmul(
                    ubp[:],
                    bmp[:, k * C : (k + 1) * C],
                    ut[k][:, bsl],
                    start=(k == 0),
                    stop=(k == 3),
                )
            ubps[bh] = ubp

        def copy_psit(bh):
            bsl = slice(bh * 512, (bh + 1) * 512)
            nc.scalar.activation(
                psit_r[:, bsl],
                pt_all[:, bsl],
                mybir.ActivationFunctionType.Copy,
            )

        # DVE chain for a bk-range is emitted as soon as its last bk's
        # stats exist; PE transposes are emitted a bk later so they hide
        # behind that bk's matmuls.
        psis = {}
        for bk in range(NBK):
            for pair in range(4, NPAIR):
                pair_mm(pair, bk)
            if bk == 2:
                hoist_ubp(0)
                psis[0] = endchain_bks(0, 3)
            if bk == 3:
                psis[1] = endchain_bks(3, 4)
            if bk == 4:
                transposes_for(psis[0], 0, 3)
                transposes_for(psis[1], 3, 4)
                copy_psit(0)
                tail_half(0)
            if bk == 5:
                psis[2] = endchain_bks(4, 6)
            if bk == 6:
                hoist_ubp(1)
                psis[3] = endchain_bks(6, 7)
        psis[4] = endchain_bks(7, 8)
        transposes_for(psis[2], 4, 6)
        transposes_for(psis[3], 6, 7)
        transposes_for(psis[4], 7, 8)
        copy_psit(1)
        tail_half(1)

    nc.compile()
    return nc


def host_prep(y, z, u, mu, sigma_inv, a_coef, b_coef, bias):
    """Host-side precompute: shared tensors + per-core input maps."""
    f64 = np.float64
    W = np.zeros((C, L, ORD), f64)
    g = np.zeros((C, L), f64)
    for c in range(C):
        a = a_coef[c].astype(f64)
        S = np.eye(ORD, dtype=f64)
        sb = np.zeros(ORD, f64)
        for l in range(L):
            ya = a @ S
            yb = a @ sb + 1.0
            W[c, l] = ya
            g[c, l] = yb
            S = np.vstack([S[1:], ya[None]])
            sb = np.concatenate([sb[1:], [yb]])
    wflat = np.ascontiguousarray(W.transpose(0, 2, 1).reshape(CO, L)).astype(np.float32)
    gmat = g.astype(np.float32)

    si = sigma_inv.astype(f64)
    m = np.einsum("cij,ci->cj", si, mu.astype(f64))   # p_c = si_c^T mu_c
    q = np.einsum("cij,cj->ci", si, m)                # q_c = si_c p_c
    k = np.sum(m * m, axis=1)                         # k_c = ||p_c||^2
    qa = q.T.astype(np.float32)                       # [D, C]

    # pk1: qa0 | qa1 | ktile64 | ebias
    pk1 = np.empty((128, 97), np.float32)
    pk1[:, 0:16] = qa[0:128]
    pk1[:, 16:32] = qa[128:256]
    pk1[:, 32:96] = np.tile(k.astype(np.float32), 4)[None, :]
    pk1[:, 96] = EXPB

    # pkw: wflat k-chunks side by side
    pkw = np.empty((128, 2 * L), np.float32)
    pkw[:, 0:L] = wflat[0:128]
    pkw[:, L : 2 * L] = wflat[128:256]

    # bmp: bmat k-chunks side by side
    bmat = np.zeros((CE, C), np.float32)
    for c in range(C):
        bmat[c * E : (c + 1) * E, c] = b_coef[c]
    bmp = np.empty((128, 4 * C), np.float32)
    for kk in range(4):
        bmp[:, kk * C : (kk + 1) * C] = bmat[kk * 128 : (kk + 1) * 128]

    # pke: emat | gmat | biasv
    emat = np.zeros((C, CO), np.float32)
    for c in range(C):
        emat[c, c * ORD : (c + 1) * ORD] = 1.0
    pke = np.empty((C, CO + L + 1), np.float32)
    pke[:, 0:CO] = emat
    pke[:, CO : CO + L] = gmat
    pke[:, CO + L] = bias.astype(np.float32)

    # sgr[i, pair, :]: ACT pairs store [sig_{2p} | sig_{2p+1}] contiguously,
    # DVE pairs interleave the two clusters' columns (2j+cc) for bn_stats.
    sit = sigma_inv.astype(np.float32).transpose(1, 0, 2)    # [i, c, j]
    sgr = np.empty((D, NPAIR, 512), np.float32)
    for p in range(NPAIR):
        if p in ACT_PAIRS:
            sgr[:, p, 0:256] = sit[:, 2 * p, :]
            sgr[:, p, 256:512] = sit[:, 2 * p + 1, :]
        else:
            sgr[:, p, 0::2] = sit[:, 2 * p, :]
            sgr[:, p, 1::2] = sit[:, 2 * p + 1, :]

    shared = {
        "sgr": sgr,
        "pk1": pk1,
        "pkw": pkw,
        "bmp": bmp,
        "pke": pke,
        "identb": np.eye(128, dtype=ml_dtypes.bfloat16),
    }
    in_maps = []
    for i in range(N_CORES):
        s = slice(i * BLOC, (i + 1) * BLOC)
        m_i = dict(shared)
        m_i["zt"] = np.ascontiguousarray(z[s, 0, :].T)
        m_i["s0t"] = np.ascontiguousarray(y[s, :, R - ORD :].reshape(BLOC, CO).T)
        m_i["ut"] = np.ascontiguousarray(u[s].reshape(BLOC, CE).T)
        in_maps.append(m_i)
    return in_maps


def kernel(y, z, u, mu, sigma_inv, a_coef, b_coef, bias, _trace=False):
    if "nc" not in _CACHE:
        _CACHE["nc"] = build_program()
    nc = _CACHE["nc"]
    in_maps = host_prep(y, z, u, mu, sigma_inv, a_coef, b_coef, bias)
    res = run_bass_kernel_spmd(
        nc, in_maps, core_ids=list(range(N_CORES)), trace=_trace
    )
    _CACHE["last_result"] = res
    out = np.concatenate(
        [res.results[i]["outT"].T[:, None, :] for i in range(N_CORES)], axis=0
    )
    return out
